# revision 18
# baseline (speedup 1.0000x reference)
"""Trainium2 Bass kernel for nn_BaseModel_91096256348406 (GRU pose net + LBS skinning).

Host side: pure relayout/sharding (transpose planes, dtype casts, weight
pre-transposes). Device side (SPMD x8 NeuronCores): 2-layer GRU encoder +
2-layer GRU decoder (single step from h0=0, so gh == bhh and h' = (1-z)*n),
output heads, then per-vertex skinning:

    out[n,d] = v[n,d]*wsum[n] + sum_k w[n,k]*off[idx[n,k],d]

The 22-entry table lookup is computed as 22 per-bone masked accumulations:
bf16 equality masks on the Vector engine (4x mode), weight product at 2x,
and a fused (S_b * off_bd + P_d) scalar_tensor_tensor accumulation in fp32.
"""

import sys

sys.path.insert(0, "/opt/trn_rl_repo")

import numpy as np
import ml_dtypes
from contextlib import ExitStack

import concourse.bass as bass
import concourse.mybir as mybir
from concourse import bacc, tile
from concourse.bass_utils import run_bass_kernel_spmd

F32 = mybir.dt.float32
BF16 = mybir.dt.bfloat16
A = mybir.AluOpType
AF = mybir.ActivationFunctionType

N_CORES = 8
N_VERTS = 4194304
VS = N_VERTS // N_CORES  # 524288 vertices per core
P = 128
F_TILE = 1024
NB = 22  # bones
K = 4    # influences per vertex
H = 512
NG = 12  # 1536 gate rows = 12 chunks of 128

# decoder layer-0 input permutation (host side): reorder dec_in from
# [prev(69) skel(66) geo(128) h0(512) h1(512)] to
# [h0 h1 skel geo prev-zeros], then drop the trailing 7 all-zero rows so the
# contraction is 10 chunks of 128 (rows 1218..1279 are zeros; the SBUF tile
# columns they multiply are memset to 0).
DEC_PERM = np.concatenate(
    [np.arange(263, 1287), np.arange(69, 263), np.arange(0, 69)]
)
DEC_KEEP = 1280


def build_program(vs=VS, f_tile=F_TILE):
    """Build the SPMD single-core Bass program for a shard of `vs` vertices."""
    assert vs % (P * f_tile) == 0
    n_tiles = vs // (P * f_tile)
    F = f_tile

    nc = bacc.Bacc(None, target_bir_lowering=False)

    # ---- external inputs ----
    vT = nc.dram_tensor("vT", [3, vs], F32, kind="ExternalInput")
    wT = nc.dram_tensor("wT", [4, vs], F32, kind="ExternalInput")
    idxT = nc.dram_tensor("idxT", [4, vs], BF16, kind="ExternalInput")

    x_enc = nc.dram_tensor("x_enc", [69, 1], F32, kind="ExternalInput")
    wih = [
        nc.dram_tensor("wih0", [69, 1536], F32, kind="ExternalInput"),
        nc.dram_tensor("wih1", [512, 1536], F32, kind="ExternalInput"),
        nc.dram_tensor("wih2", [DEC_KEEP, 1536], F32, kind="ExternalInput"),
        nc.dram_tensor("wih3", [512, 1536], F32, kind="ExternalInput"),
    ]
    b12 = [nc.dram_tensor(f"b12_{i}", [128, NG], F32, kind="ExternalInput") for i in range(4)]
    bnh = [nc.dram_tensor(f"bnh_{i}", [128, 4], F32, kind="ExternalInput") for i in range(4)]
    prefix = nc.dram_tensor("prefix", [194, 1], F32, kind="ExternalInput")
    rotwT = nc.dram_tensor("rotwT", [512, 66], F32, kind="ExternalInput")
    rotb = nc.dram_tensor("rotb", [1, 66], F32, kind="ExternalInput")
    velwT = nc.dram_tensor("velwT", [512, 3], F32, kind="ExternalInput")
    velb = nc.dram_tensor("velb", [3, 1], F32, kind="ExternalInput")
    s3 = nc.dram_tensor("s3", [3, 66], F32, kind="ExternalInput")
    ones1 = nc.dram_tensor("ones1", [1, 128], F32, kind="ExternalInput")

    # ---- external outputs ----
    outT = nc.dram_tensor("outT", [3, vs], F32, kind="ExternalOutput")
    gpos = nc.dram_tensor("gpos", [1, 66], F32, kind="ExternalOutput")

    with tile.TileContext(nc) as tc, ExitStack() as ctx:
        constp = ctx.enter_context(tc.tile_pool(name="constp", bufs=1))
        wpool = ctx.enter_context(tc.tile_pool(name="wpool", bufs=3))
        grup = ctx.enter_context(tc.tile_pool(name="grup", bufs=2))
        psum = ctx.enter_context(tc.tile_pool(name="psum", bufs=2, space="PSUM"))
        psumc = ctx.enter_context(tc.tile_pool(name="psumc", bufs=1, space="PSUM"))

        # --- small constants into SBUF ---
        xenc_t = constp.tile([128, 1], F32, tag="xenc")
        nc.sync.dma_start(xenc_t[:69, :], x_enc[:, :])
        b12_t = []
        bnh_t = []
        for i in range(4):
            bt = constp.tile([128, NG], F32, tag=f"b12_{i}")
            nc.sync.dma_start(bt[:], b12[i][:])
            b12_t.append(bt)
            nt = constp.tile([128, 4], F32, tag=f"bnh_{i}")
            nc.sync.dma_start(nt[:], bnh[i][:])
            bnh_t.append(nt)
        rotw_t = constp.tile([128, 4 * 66], F32, tag="rotw")
        for c in range(4):
            nc.sync.dma_start(rotw_t[:, c * 66:(c + 1) * 66], rotwT[c * 128:(c + 1) * 128, :])
        rotb_t = constp.tile([1, 66], F32, tag="rotb")
        nc.sync.dma_start(rotb_t[:], rotb[:])
        velw_t = constp.tile([128, 4 * 3], F32, tag="velw")
        for c in range(4):
            nc.sync.dma_start(velw_t[:, c * 3:(c + 1) * 3], velwT[c * 128:(c + 1) * 128, :])
        velb_t = constp.tile([3, 1], F32, tag="velb")
        nc.sync.dma_start(velb_t[:], velb[:])
        s3_t = constp.tile([3, 66], F32, tag="s3")
        nc.sync.dma_start(s3_t[:], s3[:])
        ones_t = constp.tile([1, 128], F32, tag="ones1")
        nc.sync.dma_start(ones_t[:], ones1[:])

        # --- GRU: one cell per layer, h_prev = 0 => gh = bhh, h' = (1-z)*n ---
        def gru_cell(layer, rhs_cols, chunk_sizes):
            n_chunks = len(chunk_sizes)
            pg = psum.tile([128, NG], F32, tag="gates", name="pg")
            for m in range(NG):
                row0 = 0
                for c in range(n_chunks):
                    C = chunk_sizes[c]
                    wt = wpool.tile([128, 128], F32, tag="wih", name="wt")
                    nc.scalar.dma_start(
                        wt[:C, :], wih[layer][row0:row0 + C, m * 128:(m + 1) * 128]
                    )
                    row0 += C
                    nc.tensor.matmul(
                        pg[:, m:m + 1],
                        wt[:C, :],
                        rhs_cols(c),
                        start=(c == 0),
                        stop=(c == n_chunks - 1),
                    )
            g = grup.tile([128, NG], F32, tag="g")
            nc.vector.tensor_tensor(g[:], pg[:], b12_t[layer][:], A.add)
            r = grup.tile([128, 4], F32, tag="r")
            nc.scalar.activation(r[:], g[:, 0:4], AF.Sigmoid)
            z = grup.tile([128, 4], F32, tag="z")
            nc.scalar.activation(z[:], g[:, 4:8], AF.Sigmoid)
            npre = grup.tile([128, 4], F32, tag="npre")
            nc.vector.tensor_tensor(npre[:], r[:], bnh_t[layer][:], A.mult)
            nc.vector.tensor_tensor(npre[:], g[:, 8:12], npre[:], A.add)
            nn = grup.tile([128, 4], F32, tag="nn")
            nc.scalar.activation(nn[:], npre[:], AF.Tanh)
            zn = grup.tile([128, 4], F32, tag="zn")
            nc.vector.tensor_tensor(zn[:], z[:], nn[:], A.mult)
            h = constp.tile([128, 4], F32, tag=f"h{layer}")
            nc.vector.tensor_tensor(h[:], nn[:], zn[:], A.subtract)
            return h

        h0 = gru_cell(0, lambda c: xenc_t[:69, :], [69])
        h1 = gru_cell(1, lambda c: h0[:, c:c + 1], [128] * 4)

        # decoder input tile [128, 10]: cols 0..3 = h_enc0, 4..7 = h_enc1,
        # col 8 = (skel|geo)[0:128], col 9 = (skel|geo)[128:194] then zeros.
        dec_t = constp.tile([128, 10], F32, tag="dec_t")
        nc.vector.tensor_copy(dec_t[:, 0:4], h0[:])
        nc.vector.tensor_copy(dec_t[:, 4:8], h1[:])
        nc.gpsimd.memset(dec_t[:, 9:10], 0.0)
        nc.sync.dma_start(dec_t[:, 8:9], prefix[0:128, :])
        nc.sync.dma_start(dec_t[:66, 9:10], prefix[128:194, :])

        hd0 = gru_cell(2, lambda c: dec_t[:, c:c + 1], [128] * 10)
        hd1 = gru_cell(3, lambda c: hd0[:, c:c + 1], [128] * 4)

        # --- heads ---
        # local_rot row [1,66] = h_last^T @ rotW^T + rotb
        lr_ps = psumc.tile([1, 66], F32, tag="lr")
        for c in range(4):
            nc.tensor.matmul(
                lr_ps[:], hd1[:, c:c + 1], rotw_t[:, c * 66:(c + 1) * 66],
                start=(c == 0), stop=(c == 3),
            )
        lr_row = constp.tile([1, 66], F32, tag="lr_row")
        nc.vector.tensor_tensor(lr_row[:], lr_ps[:], rotb_t[:], A.add)

        # root_vel column [3,1] = (velW^T)^T @ h_last + velb
        rv_ps = psumc.tile([3, 1], F32, tag="rv")
        for c in range(4):
            nc.tensor.matmul(
                rv_ps[:], velw_t[:, c * 3:(c + 1) * 3], hd1[:, c:c + 1],
                start=(c == 0), stop=(c == 3),
            )
        rv_col = constp.tile([3, 1], F32, tag="rv_col")
        nc.vector.tensor_tensor(rv_col[:], rv_ps[:], velb_t[:], A.add)

        # rv repeated 22x: [1,66] = rv_col^T @ s3
        rv66_ps = psumc.tile([1, 66], F32, tag="rv66")
        nc.tensor.matmul(rv66_ps[:], rv_col[:], s3_t[:], start=True, stop=True)
        gp_row = constp.tile([1, 66], F32, tag="gp_row")
        nc.vector.tensor_tensor(gp_row[:], lr_row[:], rv66_ps[:], A.mult)
        nc.sync.dma_start(gpos[:], gp_row[:])

        # offsets broadcast to all partitions: [128, 66] = ones^T @ lr_row
        offc_ps = psumc.tile([128, 66], F32, tag="offc")
        nc.tensor.matmul(offc_ps[:], ones_t[:], lr_row[:], start=True, stop=True)
        offc = constp.tile([128, 66], F32, tag="offc")
        nc.vector.tensor_copy(offc[:], offc_ps[:])

        # --- skinning over vertex tiles ---
        vstream = ctx.enter_context(tc.tile_pool(name="vstream", bufs=2))
        wbfp = ctx.enter_context(tc.tile_pool(name="wbfp", bufs=1))
        maskp = ctx.enter_context(tc.tile_pool(name="maskp", bufs=4))
        mwp = ctx.enter_context(tc.tile_pool(name="mwp", bufs=6))
        streep = ctx.enter_context(tc.tile_pool(name="streep", bufs=4))
        accp = ctx.enter_context(tc.tile_pool(name="accp", bufs=1))
        outp = ctx.enter_context(tc.tile_pool(name="outp", bufs=2))

        for t in range(n_tiles):
            base = t * P * F
            sl = lambda: slice(base, base + P * F)

            idx_t = [vstream.tile([P, F], BF16, tag=f"idx{k}", name=f"idx{k}") for k in range(K)]
            w_t = [vstream.tile([P, F], F32, tag=f"w{k}", name=f"w{k}") for k in range(K)]
            v_t = [vstream.tile([P, F], F32, tag=f"v{d}", name=f"v{d}") for d in range(3)]
            for k in range(K):
                nc.sync.dma_start(idx_t[k][:], idxT[k, sl()].rearrange("(p f) -> p f", p=P))
                nc.sync.dma_start(w_t[k][:], wT[k, sl()].rearrange("(p f) -> p f", p=P))
            for d in range(3):
                nc.sync.dma_start(v_t[d][:], vT[d, sl()].rearrange("(p f) -> p f", p=P))

            # bf16 copies of w on the Scalar engine (frees DVE cycles)
            wbf_t = [wbfp.tile([P, F], BF16, tag=f"wbf{k}", name=f"wbf{k}") for k in range(K)]
            for k in range(K):
                nc.scalar.copy(wbf_t[k][:], w_t[k][:])

            # wsum in fp32
            ws01 = accp.tile([P, F], F32, tag="ws01")
            ws = accp.tile([P, F], F32, tag="ws")
            nc.vector.tensor_tensor(ws01[:], w_t[0][:], w_t[1][:], A.add)
            nc.vector.tensor_tensor(ws[:], w_t[2][:], w_t[3][:], A.add)
            nc.vector.tensor_tensor(ws[:], ws01[:], ws[:], A.add)

            p_t = [accp.tile([P, F], F32, tag=f"P{d}", name=f"P{d}") for d in range(3)]
            for b in range(NB):
                mw = []
                for k in range(K):
                    m = maskp.tile([P, F], BF16, tag="mask")
                    nc.vector.tensor_scalar(m[:], idx_t[k][:], float(b), None, A.is_equal)
                    mwk = mwp.tile([P, F], BF16, tag="mw")
                    nc.vector.tensor_tensor(mwk[:], m[:], wbf_t[k][:], A.mult)
                    mw.append(mwk)
                s01 = streep.tile([P, F], BF16, tag="s01")
                s23 = streep.tile([P, F], BF16, tag="s23")
                sb = streep.tile([P, F], BF16, tag="sb")
                nc.vector.tensor_tensor(s01[:], mw[0][:], mw[1][:], A.add)
                nc.vector.tensor_tensor(s23[:], mw[2][:], mw[3][:], A.add)
                nc.vector.tensor_tensor(sb[:], s01[:], s23[:], A.add)
                for d in range(3):
                    col = offc[:, 3 * b + d:3 * b + d + 1]
                    if b == 0:
                        nc.vector.tensor_scalar(p_t[d][:], sb[:], col, None, A.mult)
                    else:
                        nc.vector.scalar_tensor_tensor(
                            p_t[d][:], sb[:], col, p_t[d][:], A.mult, A.add
                        )

            for d in range(3):
                o = outp.tile([P, F], F32, tag=f"o{d}")
                nc.vector.tensor_tensor(o[:], v_t[d][:], ws[:], A.mult)
                nc.vector.tensor_tensor(o[:], o[:], p_t[d][:], A.add)
                nc.gpsimd.dma_start(outT[d, sl()].rearrange("(p f) -> p f", p=P), o[:])

    nc.compile()
    return nc


def prep_shared_inputs(rotation, velocity, params):
    """Host-side relayout of the small inputs and parameters (shared by all cores)."""
    f32 = lambda a: np.ascontiguousarray(np.asarray(a, dtype=np.float32))
    rotation = f32(rotation)
    velocity = f32(velocity)

    enc = params["enc"]
    dec = params["dec"]
    layers = [enc[0], enc[1], dec[0], dec[1]]

    shared = {}
    shared["x_enc"] = np.concatenate([rotation.reshape(-1), velocity]).astype(np.float32).reshape(69, 1)

    for i, (Wih, Whh, bih, bhh) in enumerate(layers):
        Wt = f32(Wih).T  # [in, 1536]
        if i == 2:
            Wt = Wt[DEC_PERM][:DEC_KEEP]
        shared[f"wih{i}"] = np.ascontiguousarray(Wt)
        bih = f32(bih)
        bhh = f32(bhh)
        vec = np.concatenate([(bih + bhh)[:1024], bih[1024:]])
        shared[f"b12_{i}"] = np.ascontiguousarray(vec.reshape(NG, 128).T)
        shared[f"bnh_{i}"] = np.ascontiguousarray(bhh[1024:].reshape(4, 128).T)

    shared["prefix"] = np.concatenate(
        [f32(params["skel"]).reshape(-1), f32(params["geo_emb"])]
    ).astype(np.float32).reshape(194, 1)
    shared["rotwT"] = np.ascontiguousarray(f32(params["rotW"]).T)
    shared["rotb"] = f32(params["rotb"]).reshape(1, 66)
    shared["velwT"] = np.ascontiguousarray(f32(params["velW"]).T)
    shared["velb"] = f32(params["velb"]).reshape(3, 1)
    s3 = np.zeros((3, 66), np.float32)
    s3[np.arange(66) % 3, np.arange(66)] = 1.0
    shared["s3"] = s3
    shared["ones1"] = np.ones((1, 128), np.float32)
    return shared


def prep_shard(vertices, skin_idx, skin_w, s, vs):
    sl = slice(s * vs, (s + 1) * vs)
    return {
        "vT": np.ascontiguousarray(np.asarray(vertices[sl], np.float32).T),
        "wT": np.ascontiguousarray(np.asarray(skin_w[sl], np.float32).T),
        "idxT": np.ascontiguousarray(
            np.asarray(skin_idx[sl]).T.astype(ml_dtypes.bfloat16)
        ),
    }


LAST_RESULTS = None
_NC_CACHE = {}


def _get_program(vs, f_tile):
    key = (vs, f_tile)
    if key not in _NC_CACHE:
        _NC_CACHE[key] = build_program(vs, f_tile)
    return _NC_CACHE[key]


def kernel(rotation, velocity, vertices, skin_idx, skin_w, params):
    vertices = np.asarray(vertices)
    skin_idx = np.asarray(skin_idx)
    skin_w = np.asarray(skin_w)
    n = vertices.shape[0]
    vs = n // N_CORES

    shared = prep_shared_inputs(rotation, velocity, params)
    in_maps = []
    for s in range(N_CORES):
        m = dict(shared)
        m.update(prep_shard(vertices, skin_idx, skin_w, s, vs))
        in_maps.append(m)

    nc = _get_program(vs, F_TILE)
    import os

    res = run_bass_kernel_spmd(
        nc, in_maps, list(range(N_CORES)),
        trace=bool(os.environ.get("KERNEL_TRACE")),
    )
    global LAST_RESULTS
    LAST_RESULTS = res
    outs = res.results

    vertex_positions = np.concatenate(
        [np.ascontiguousarray(outs[s]["outT"].T) for s in range(N_CORES)], axis=0
    ).astype(np.float32)
    global_positions = np.asarray(outs[0]["gpos"], np.float32).reshape(66)
    return (global_positions, vertex_positions)


# revision 24
# speedup vs baseline: 1.1534x; 1.1534x over previous
"""Trainium2 Bass kernel for nn_BaseModel_91096256348406 (GRU pose net + LBS skinning).

Host side: pure relayout/sharding (transpose planes, dtype casts, weight
pre-transposes). Device side (SPMD x8 NeuronCores): 2-layer GRU encoder +
2-layer GRU decoder (single step from h0=0, so gh == bhh and h' = (1-z)*n),
output heads, then per-vertex skinning:

    out[n,d] = v[n,d]*wsum[n] + sum_k w[n,k]*off[idx[n,k],d]

The 22-entry table lookup is computed as 22 per-bone masked accumulations:
bf16 equality masks on the Vector engine (4x mode), weight product at 2x,
and a fused (S_b * off_bd + P_d) scalar_tensor_tensor accumulation in fp32.
"""

import sys

sys.path.insert(0, "/opt/trn_rl_repo")

import numpy as np
import ml_dtypes
from contextlib import ExitStack

import concourse.bass as bass
import concourse.mybir as mybir
from concourse import bacc, tile
from concourse.bass_utils import run_bass_kernel_spmd

F32 = mybir.dt.float32
BF16 = mybir.dt.bfloat16
A = mybir.AluOpType
AF = mybir.ActivationFunctionType

N_CORES = 8
N_VERTS = 4194304
VS = N_VERTS // N_CORES  # 524288 vertices per core
P = 128
F_TILE = 1024
NB = 22  # bones
K = 4    # influences per vertex
H = 512
NG = 12  # 1536 gate rows = 12 chunks of 128

# decoder layer-0 input permutation (host side): reorder dec_in from
# [prev(69) skel(66) geo(128) h0(512) h1(512)] to
# [h0 h1 skel geo prev-zeros], then drop the trailing 7 all-zero rows so the
# contraction is 10 chunks of 128 (rows 1218..1279 are zeros; the SBUF tile
# columns they multiply are memset to 0).
DEC_PERM = np.concatenate(
    [np.arange(263, 1287), np.arange(69, 263), np.arange(0, 69)]
)
DEC_KEEP = 1280


def build_program(vs=VS, f_tile=F_TILE):
    """Build the SPMD single-core Bass program for a shard of `vs` vertices."""
    assert vs % (P * f_tile) == 0
    n_tiles = vs // (P * f_tile)
    F = f_tile

    nc = bacc.Bacc(None, target_bir_lowering=False)

    # ---- external inputs ----
    vT = nc.dram_tensor("vT", [3, vs], F32, kind="ExternalInput")
    wT = nc.dram_tensor("wT", [4, vs], F32, kind="ExternalInput")
    idxT = nc.dram_tensor("idxT", [4, vs], BF16, kind="ExternalInput")

    x_enc = nc.dram_tensor("x_enc", [69, 1], F32, kind="ExternalInput")
    wih = [
        nc.dram_tensor("wih0", [69, 1536], F32, kind="ExternalInput"),
        nc.dram_tensor("wih1", [512, 1536], F32, kind="ExternalInput"),
        nc.dram_tensor("wih2", [DEC_KEEP, 1536], F32, kind="ExternalInput"),
        nc.dram_tensor("wih3", [512, 1536], F32, kind="ExternalInput"),
    ]
    b12 = [nc.dram_tensor(f"b12_{i}", [128, NG], F32, kind="ExternalInput") for i in range(4)]
    bnh = [nc.dram_tensor(f"bnh_{i}", [128, 4], F32, kind="ExternalInput") for i in range(4)]
    prefix = nc.dram_tensor("prefix", [194, 1], F32, kind="ExternalInput")
    rotwT = nc.dram_tensor("rotwT", [512, 66], F32, kind="ExternalInput")
    rotb = nc.dram_tensor("rotb", [1, 66], F32, kind="ExternalInput")
    velwT = nc.dram_tensor("velwT", [512, 3], F32, kind="ExternalInput")
    velb = nc.dram_tensor("velb", [3, 1], F32, kind="ExternalInput")
    s3 = nc.dram_tensor("s3", [3, 66], F32, kind="ExternalInput")
    ones1 = nc.dram_tensor("ones1", [1, 128], F32, kind="ExternalInput")

    # ---- external outputs ----
    outT = nc.dram_tensor("outT", [3, vs], F32, kind="ExternalOutput")
    gpos = nc.dram_tensor("gpos", [1, 66], F32, kind="ExternalOutput")

    with tile.TileContext(nc) as tc, ExitStack() as ctx:
        constp = ctx.enter_context(tc.tile_pool(name="constp", bufs=1))
        wpool = ctx.enter_context(tc.tile_pool(name="wpool", bufs=3))
        grup = ctx.enter_context(tc.tile_pool(name="grup", bufs=2))
        psum = ctx.enter_context(tc.tile_pool(name="psum", bufs=2, space="PSUM"))
        psumc = ctx.enter_context(tc.tile_pool(name="psumc", bufs=1, space="PSUM"))

        # --- small constants into SBUF ---
        xenc_t = constp.tile([128, 1], F32, tag="xenc")
        nc.sync.dma_start(xenc_t[:69, :], x_enc[:, :])
        b12_t = []
        bnh_t = []
        for i in range(4):
            bt = constp.tile([128, NG], F32, tag=f"b12_{i}")
            nc.sync.dma_start(bt[:], b12[i][:])
            b12_t.append(bt)
            nt = constp.tile([128, 4], F32, tag=f"bnh_{i}")
            nc.sync.dma_start(nt[:], bnh[i][:])
            bnh_t.append(nt)
        rotw_t = constp.tile([128, 4 * 66], F32, tag="rotw")
        for c in range(4):
            nc.sync.dma_start(rotw_t[:, c * 66:(c + 1) * 66], rotwT[c * 128:(c + 1) * 128, :])
        rotb_t = constp.tile([1, 66], F32, tag="rotb")
        nc.sync.dma_start(rotb_t[:], rotb[:])
        velw_t = constp.tile([128, 4 * 3], F32, tag="velw")
        for c in range(4):
            nc.sync.dma_start(velw_t[:, c * 3:(c + 1) * 3], velwT[c * 128:(c + 1) * 128, :])
        velb_t = constp.tile([3, 1], F32, tag="velb")
        nc.sync.dma_start(velb_t[:], velb[:])
        s3_t = constp.tile([3, 66], F32, tag="s3")
        nc.sync.dma_start(s3_t[:], s3[:])
        ones_t = constp.tile([1, 128], F32, tag="ones1")
        nc.sync.dma_start(ones_t[:], ones1[:])

        # --- GRU: one cell per layer, h_prev = 0 => gh = bhh, h' = (1-z)*n ---
        def gru_cell(layer, rhs_cols, chunk_sizes):
            n_chunks = len(chunk_sizes)
            pg = psum.tile([128, NG], F32, tag="gates", name="pg")
            for m in range(NG):
                row0 = 0
                for c in range(n_chunks):
                    C = chunk_sizes[c]
                    wt = wpool.tile([128, 128], F32, tag="wih", name="wt")
                    nc.scalar.dma_start(
                        wt[:C, :], wih[layer][row0:row0 + C, m * 128:(m + 1) * 128]
                    )
                    row0 += C
                    nc.tensor.matmul(
                        pg[:, m:m + 1],
                        wt[:C, :],
                        rhs_cols(c),
                        start=(c == 0),
                        stop=(c == n_chunks - 1),
                    )
            g = grup.tile([128, NG], F32, tag="g")
            nc.vector.tensor_tensor(g[:], pg[:], b12_t[layer][:], A.add)
            r = grup.tile([128, 4], F32, tag="r")
            nc.scalar.activation(r[:], g[:, 0:4], AF.Sigmoid)
            z = grup.tile([128, 4], F32, tag="z")
            nc.scalar.activation(z[:], g[:, 4:8], AF.Sigmoid)
            npre = grup.tile([128, 4], F32, tag="npre")
            nc.vector.tensor_tensor(npre[:], r[:], bnh_t[layer][:], A.mult)
            nc.vector.tensor_tensor(npre[:], g[:, 8:12], npre[:], A.add)
            nn = grup.tile([128, 4], F32, tag="nn")
            nc.scalar.activation(nn[:], npre[:], AF.Tanh)
            zn = grup.tile([128, 4], F32, tag="zn")
            nc.vector.tensor_tensor(zn[:], z[:], nn[:], A.mult)
            h = constp.tile([128, 4], F32, tag=f"h{layer}")
            nc.vector.tensor_tensor(h[:], nn[:], zn[:], A.subtract)
            return h

        h0 = gru_cell(0, lambda c: xenc_t[:69, :], [69])
        h1 = gru_cell(1, lambda c: h0[:, c:c + 1], [128] * 4)

        # decoder input tile [128, 10]: cols 0..3 = h_enc0, 4..7 = h_enc1,
        # col 8 = (skel|geo)[0:128], col 9 = (skel|geo)[128:194] then zeros.
        dec_t = constp.tile([128, 10], F32, tag="dec_t")
        nc.vector.tensor_copy(dec_t[:, 0:4], h0[:])
        nc.vector.tensor_copy(dec_t[:, 4:8], h1[:])
        nc.gpsimd.memset(dec_t[:, 9:10], 0.0)
        nc.sync.dma_start(dec_t[:, 8:9], prefix[0:128, :])
        nc.sync.dma_start(dec_t[:66, 9:10], prefix[128:194, :])

        hd0 = gru_cell(2, lambda c: dec_t[:, c:c + 1], [128] * 10)
        hd1 = gru_cell(3, lambda c: hd0[:, c:c + 1], [128] * 4)

        # --- heads ---
        # local_rot row [1,66] = h_last^T @ rotW^T + rotb
        lr_ps = psumc.tile([1, 66], F32, tag="lr")
        for c in range(4):
            nc.tensor.matmul(
                lr_ps[:], hd1[:, c:c + 1], rotw_t[:, c * 66:(c + 1) * 66],
                start=(c == 0), stop=(c == 3),
            )
        lr_row = constp.tile([1, 66], F32, tag="lr_row")
        nc.vector.tensor_tensor(lr_row[:], lr_ps[:], rotb_t[:], A.add)

        # root_vel column [3,1] = (velW^T)^T @ h_last + velb
        rv_ps = psumc.tile([3, 1], F32, tag="rv")
        for c in range(4):
            nc.tensor.matmul(
                rv_ps[:], velw_t[:, c * 3:(c + 1) * 3], hd1[:, c:c + 1],
                start=(c == 0), stop=(c == 3),
            )
        rv_col = constp.tile([3, 1], F32, tag="rv_col")
        nc.vector.tensor_tensor(rv_col[:], rv_ps[:], velb_t[:], A.add)

        # rv repeated 22x: [1,66] = rv_col^T @ s3
        rv66_ps = psumc.tile([1, 66], F32, tag="rv66")
        nc.tensor.matmul(rv66_ps[:], rv_col[:], s3_t[:], start=True, stop=True)
        gp_row = constp.tile([1, 66], F32, tag="gp_row")
        nc.vector.tensor_tensor(gp_row[:], lr_row[:], rv66_ps[:], A.mult)
        nc.sync.dma_start(gpos[:], gp_row[:])

        # offsets broadcast to all partitions: [128, 66] = ones^T @ lr_row
        offc_ps = psumc.tile([128, 66], F32, tag="offc")
        nc.tensor.matmul(offc_ps[:], ones_t[:], lr_row[:], start=True, stop=True)
        offc = constp.tile([128, 66], F32, tag="offc")
        nc.vector.tensor_copy(offc[:], offc_ps[:])

        # --- skinning over vertex tiles ---
        vstream = ctx.enter_context(tc.tile_pool(name="vstream", bufs=2))
        wbfp = ctx.enter_context(tc.tile_pool(name="wbfp", bufs=1))
        maskp = ctx.enter_context(tc.tile_pool(name="maskp", bufs=4))
        mwp = ctx.enter_context(tc.tile_pool(name="mwp", bufs=6))
        streep = ctx.enter_context(tc.tile_pool(name="streep", bufs=4))
        accp = ctx.enter_context(tc.tile_pool(name="accp", bufs=1))
        tscp = ctx.enter_context(tc.tile_pool(name="tscp", bufs=3))
        outp = ctx.enter_context(tc.tile_pool(name="outp", bufs=2))

        for t in range(n_tiles):
            base = t * P * F
            sl = lambda: slice(base, base + P * F)

            idx_t = [vstream.tile([P, F], BF16, tag=f"idx{k}", name=f"idx{k}") for k in range(K)]
            w_t = [vstream.tile([P, F], F32, tag=f"w{k}", name=f"w{k}") for k in range(K)]
            v_t = [vstream.tile([P, F], F32, tag=f"v{d}", name=f"v{d}") for d in range(3)]
            for k in range(K):
                nc.sync.dma_start(idx_t[k][:], idxT[k, sl()].rearrange("(p f) -> p f", p=P))
                nc.sync.dma_start(w_t[k][:], wT[k, sl()].rearrange("(p f) -> p f", p=P))
            for d in range(3):
                nc.sync.dma_start(v_t[d][:], vT[d, sl()].rearrange("(p f) -> p f", p=P))

            # bf16 copies of w on the Scalar engine (frees DVE cycles)
            wbf_t = [wbfp.tile([P, F], BF16, tag=f"wbf{k}", name=f"wbf{k}") for k in range(K)]
            for k in range(K):
                nc.scalar.copy(wbf_t[k][:], w_t[k][:])

            # wsum in fp32
            ws01 = accp.tile([P, F], F32, tag="ws01")
            ws = accp.tile([P, F], F32, tag="ws")
            nc.gpsimd.tensor_tensor(ws01[:], w_t[0][:], w_t[1][:], A.add)
            nc.gpsimd.tensor_tensor(ws[:], w_t[2][:], w_t[3][:], A.add)
            nc.gpsimd.tensor_tensor(ws[:], ws01[:], ws[:], A.add)

            p_t = [accp.tile([P, F], BF16, tag=f"P{d}", name=f"P{d}") for d in range(3)]
            for b in range(NB):
                mw = []
                for k in range(K):
                    m = maskp.tile([P, F], BF16, tag="mask")
                    nc.vector.tensor_scalar(m[:], idx_t[k][:], float(b), None, A.is_equal)
                    mwk = mwp.tile([P, F], BF16, tag="mw")
                    eng = nc.gpsimd if k == 3 else nc.vector
                    eng.tensor_tensor(mwk[:], m[:], wbf_t[k][:], A.mult)
                    mw.append(mwk)
                s01 = streep.tile([P, F], BF16, tag="s01")
                s23 = streep.tile([P, F], BF16, tag="s23")
                sb = streep.tile([P, F], BF16, tag="sb")
                nc.vector.tensor_tensor(s01[:], mw[0][:], mw[1][:], A.add)
                nc.vector.tensor_tensor(s23[:], mw[2][:], mw[3][:], A.add)
                nc.vector.tensor_tensor(sb[:], s01[:], s23[:], A.add)
                for d in range(3):
                    col = offc[:, 3 * b + d:3 * b + d + 1]
                    if b == 0:
                        nc.scalar.activation(p_t[d][:], sb[:], AF.Copy, scale=col)
                    else:
                        tsc = tscp.tile([P, F], BF16, tag=f"tsc{d}", name=f"tsc{d}")
                        nc.scalar.activation(tsc[:], sb[:], AF.Copy, scale=col)
                        nc.vector.tensor_tensor(p_t[d][:], p_t[d][:], tsc[:], A.add)

            for d in range(3):
                o = outp.tile([P, F], F32, tag=f"o{d}")
                nc.vector.tensor_tensor(o[:], v_t[d][:], ws[:], A.mult)
                nc.vector.tensor_tensor(o[:], o[:], p_t[d][:], A.add)
                nc.gpsimd.dma_start(outT[d, sl()].rearrange("(p f) -> p f", p=P), o[:])

    nc.compile()
    return nc


def prep_shared_inputs(rotation, velocity, params):
    """Host-side relayout of the small inputs and parameters (shared by all cores)."""
    f32 = lambda a: np.ascontiguousarray(np.asarray(a, dtype=np.float32))
    rotation = f32(rotation)
    velocity = f32(velocity)

    enc = params["enc"]
    dec = params["dec"]
    layers = [enc[0], enc[1], dec[0], dec[1]]

    shared = {}
    shared["x_enc"] = np.concatenate([rotation.reshape(-1), velocity]).astype(np.float32).reshape(69, 1)

    for i, (Wih, Whh, bih, bhh) in enumerate(layers):
        Wt = f32(Wih).T  # [in, 1536]
        if i == 2:
            Wt = Wt[DEC_PERM][:DEC_KEEP]
        shared[f"wih{i}"] = np.ascontiguousarray(Wt)
        bih = f32(bih)
        bhh = f32(bhh)
        vec = np.concatenate([(bih + bhh)[:1024], bih[1024:]])
        shared[f"b12_{i}"] = np.ascontiguousarray(vec.reshape(NG, 128).T)
        shared[f"bnh_{i}"] = np.ascontiguousarray(bhh[1024:].reshape(4, 128).T)

    shared["prefix"] = np.concatenate(
        [f32(params["skel"]).reshape(-1), f32(params["geo_emb"])]
    ).astype(np.float32).reshape(194, 1)
    shared["rotwT"] = np.ascontiguousarray(f32(params["rotW"]).T)
    shared["rotb"] = f32(params["rotb"]).reshape(1, 66)
    shared["velwT"] = np.ascontiguousarray(f32(params["velW"]).T)
    shared["velb"] = f32(params["velb"]).reshape(3, 1)
    s3 = np.zeros((3, 66), np.float32)
    s3[np.arange(66) % 3, np.arange(66)] = 1.0
    shared["s3"] = s3
    shared["ones1"] = np.ones((1, 128), np.float32)
    return shared


def prep_shard(vertices, skin_idx, skin_w, s, vs):
    sl = slice(s * vs, (s + 1) * vs)
    return {
        "vT": np.ascontiguousarray(np.asarray(vertices[sl], np.float32).T),
        "wT": np.ascontiguousarray(np.asarray(skin_w[sl], np.float32).T),
        "idxT": np.ascontiguousarray(
            np.asarray(skin_idx[sl]).T.astype(ml_dtypes.bfloat16)
        ),
    }


LAST_RESULTS = None
_NC_CACHE = {}


def _get_program(vs, f_tile):
    key = (vs, f_tile)
    if key not in _NC_CACHE:
        _NC_CACHE[key] = build_program(vs, f_tile)
    return _NC_CACHE[key]


def kernel(rotation, velocity, vertices, skin_idx, skin_w, params):
    vertices = np.asarray(vertices)
    skin_idx = np.asarray(skin_idx)
    skin_w = np.asarray(skin_w)
    n = vertices.shape[0]
    vs = n // N_CORES

    shared = prep_shared_inputs(rotation, velocity, params)
    in_maps = []
    for s in range(N_CORES):
        m = dict(shared)
        m.update(prep_shard(vertices, skin_idx, skin_w, s, vs))
        in_maps.append(m)

    nc = _get_program(vs, F_TILE)
    import os

    res = run_bass_kernel_spmd(
        nc, in_maps, list(range(N_CORES)),
        trace=bool(os.environ.get("KERNEL_TRACE")),
    )
    global LAST_RESULTS
    LAST_RESULTS = res
    outs = res.results

    vertex_positions = np.concatenate(
        [np.ascontiguousarray(outs[s]["outT"].T) for s in range(N_CORES)], axis=0
    ).astype(np.float32)
    global_positions = np.asarray(outs[0]["gpos"], np.float32).reshape(66)
    return (global_positions, vertex_positions)


# revision 28
# speedup vs baseline: 1.3364x; 1.1587x over previous
"""Trainium2 Bass kernel for nn_BaseModel_91096256348406 (GRU pose net + LBS skinning).

Host side: pure relayout/sharding (transpose planes, dtype casts, weight
pre-transposes). Device side (SPMD x8 NeuronCores): 2-layer GRU encoder +
2-layer GRU decoder (single step from h0=0, so gh == bhh and h' = (1-z)*n),
output heads, then per-vertex skinning:

    out[n,d] = v[n,d]*wsum[n] + sum_k w[n,k]*off[idx[n,k],d]

The 22-entry table lookup is computed as 22 per-bone masked accumulations,
split across three engines: bf16 equality masks (4x) and weight products /
pair-sums (2x) on the Vector engine, the per-bone scale S_b*off_bd on the
Scalar engine (ACT Copy with per-partition scale), the bf16 P accumulation
adds back on Vector at 2x, and wsum plus k=3's weight product on GPSIMD.
scalar_tensor_tensor was measured 1x-only and TensorScalarPtr is illegal on
Pool, which is what drove this split (see TimelineSim experiments).
"""

import sys

sys.path.insert(0, "/opt/trn_rl_repo")

import numpy as np
import ml_dtypes
from contextlib import ExitStack

import concourse.bass as bass
import concourse.mybir as mybir
from concourse import bacc, tile
from concourse.bass_utils import run_bass_kernel_spmd

F32 = mybir.dt.float32
BF16 = mybir.dt.bfloat16
A = mybir.AluOpType
AF = mybir.ActivationFunctionType

N_CORES = 8
N_VERTS = 4194304
VS = N_VERTS // N_CORES  # 524288 vertices per core
P = 128
F_TILE = 1024
NB = 22  # bones
K = 4    # influences per vertex
H = 512
NG = 12  # 1536 gate rows = 12 chunks of 128

# decoder layer-0 input permutation (host side): reorder dec_in from
# [prev(69) skel(66) geo(128) h0(512) h1(512)] to
# [h0 h1 skel geo prev-zeros], then drop the trailing 7 all-zero rows so the
# contraction is 10 chunks of 128 (rows 1218..1279 are zeros; the SBUF tile
# columns they multiply are memset to 0).
DEC_PERM = np.concatenate(
    [np.arange(263, 1287), np.arange(69, 263), np.arange(0, 69)]
)
DEC_KEEP = 1280


def build_program(vs=VS, f_tile=F_TILE):
    """Build the SPMD single-core Bass program for a shard of `vs` vertices."""
    assert vs % (P * f_tile) == 0
    n_tiles = vs // (P * f_tile)
    F = f_tile

    nc = bacc.Bacc(None, target_bir_lowering=False)

    # ---- external inputs ----
    vT = nc.dram_tensor("vT", [3, vs], F32, kind="ExternalInput")
    wT = nc.dram_tensor("wT", [4, vs], F32, kind="ExternalInput")
    idxT = nc.dram_tensor("idxT", [4, vs], BF16, kind="ExternalInput")

    x_enc = nc.dram_tensor("x_enc", [69, 1], F32, kind="ExternalInput")
    wih = [
        nc.dram_tensor("wih0", [69, 1536], F32, kind="ExternalInput"),
        nc.dram_tensor("wih1", [512, 1536], F32, kind="ExternalInput"),
        nc.dram_tensor("wih2", [DEC_KEEP, 1536], F32, kind="ExternalInput"),
        nc.dram_tensor("wih3", [512, 1536], F32, kind="ExternalInput"),
    ]
    b12 = [nc.dram_tensor(f"b12_{i}", [128, NG], F32, kind="ExternalInput") for i in range(4)]
    bnh = [nc.dram_tensor(f"bnh_{i}", [128, 4], F32, kind="ExternalInput") for i in range(4)]
    prefix = nc.dram_tensor("prefix", [194, 1], F32, kind="ExternalInput")
    rotwT = nc.dram_tensor("rotwT", [512, 66], F32, kind="ExternalInput")
    rotb = nc.dram_tensor("rotb", [1, 66], F32, kind="ExternalInput")
    velwT = nc.dram_tensor("velwT", [512, 3], F32, kind="ExternalInput")
    velb = nc.dram_tensor("velb", [3, 1], F32, kind="ExternalInput")
    s3 = nc.dram_tensor("s3", [3, 66], F32, kind="ExternalInput")
    ones1 = nc.dram_tensor("ones1", [1, 128], F32, kind="ExternalInput")

    # ---- external outputs ----
    outT = nc.dram_tensor("outT", [3, vs], F32, kind="ExternalOutput")
    gpos = nc.dram_tensor("gpos", [1, 66], F32, kind="ExternalOutput")

    with tile.TileContext(nc) as tc, ExitStack() as ctx:
        constp = ctx.enter_context(tc.tile_pool(name="constp", bufs=1))
        wpool = ctx.enter_context(tc.tile_pool(name="wpool", bufs=11))
        grup = ctx.enter_context(tc.tile_pool(name="grup", bufs=2))
        psum = ctx.enter_context(tc.tile_pool(name="psum", bufs=2, space="PSUM"))
        psumc = ctx.enter_context(tc.tile_pool(name="psumc", bufs=1, space="PSUM"))

        # --- small constants into SBUF ---
        xenc_t = constp.tile([128, 1], F32, tag="xenc")
        nc.sync.dma_start(xenc_t[:69, :], x_enc[:, :])
        b12_t = []
        bnh_t = []
        for i in range(4):
            bt = constp.tile([128, NG], F32, tag=f"b12_{i}")
            nc.sync.dma_start(bt[:], b12[i][:])
            b12_t.append(bt)
            nt = constp.tile([128, 4], F32, tag=f"bnh_{i}")
            nc.sync.dma_start(nt[:], bnh[i][:])
            bnh_t.append(nt)
        rotw_t = constp.tile([128, 4 * 66], F32, tag="rotw")
        for c in range(4):
            nc.sync.dma_start(rotw_t[:, c * 66:(c + 1) * 66], rotwT[c * 128:(c + 1) * 128, :])
        rotb_t = constp.tile([1, 66], F32, tag="rotb")
        nc.sync.dma_start(rotb_t[:], rotb[:])
        velw_t = constp.tile([128, 4 * 3], F32, tag="velw")
        for c in range(4):
            nc.sync.dma_start(velw_t[:, c * 3:(c + 1) * 3], velwT[c * 128:(c + 1) * 128, :])
        velb_t = constp.tile([3, 1], F32, tag="velb")
        nc.sync.dma_start(velb_t[:], velb[:])
        s3_t = constp.tile([3, 66], F32, tag="s3")
        nc.sync.dma_start(s3_t[:], s3[:])
        ones_t = constp.tile([1, 128], F32, tag="ones1")
        nc.sync.dma_start(ones_t[:], ones1[:])

        # --- GRU: one cell per layer, h_prev = 0 => gh = bhh, h' = (1-z)*n ---
        def gru_cell(layer, rhs_cols, chunk_sizes):
            n_chunks = len(chunk_sizes)
            pg = psum.tile([128, NG], F32, tag="gates", name="pg")
            # 512-col weight slabs: 1 DMA per (chunk, slab) instead of per
            # (chunk, m) — 57 total issues instead of 228. m loops within a
            # resident slab so each PSUM column's accumulation group closes
            # before the next opens.
            for g in range(3):
                slabs = []
                row0 = 0
                for c in range(n_chunks):
                    C = chunk_sizes[c]
                    st = wpool.tile([128, 512], F32, tag="wslab", name="wslab")
                    nc.sync.dma_start(
                        st[:C, :], wih[layer][row0:row0 + C, g * 512:(g + 1) * 512]
                    )
                    row0 += C
                    slabs.append((st, C))
                for mi in range(4):
                    m = g * 4 + mi
                    for c in range(n_chunks):
                        st, C = slabs[c]
                        nc.tensor.matmul(
                            pg[:, m:m + 1],
                            st[:C, mi * 128:(mi + 1) * 128],
                            rhs_cols(c),
                            start=(c == 0),
                            stop=(c == n_chunks - 1),
                        )
            g = grup.tile([128, NG], F32, tag="g")
            nc.vector.tensor_tensor(g[:], pg[:], b12_t[layer][:], A.add)
            r = grup.tile([128, 4], F32, tag="r")
            nc.scalar.activation(r[:], g[:, 0:4], AF.Sigmoid)
            z = grup.tile([128, 4], F32, tag="z")
            nc.scalar.activation(z[:], g[:, 4:8], AF.Sigmoid)
            npre = grup.tile([128, 4], F32, tag="npre")
            nc.vector.tensor_tensor(npre[:], r[:], bnh_t[layer][:], A.mult)
            nc.vector.tensor_tensor(npre[:], g[:, 8:12], npre[:], A.add)
            nn = grup.tile([128, 4], F32, tag="nn")
            nc.scalar.activation(nn[:], npre[:], AF.Tanh)
            zn = grup.tile([128, 4], F32, tag="zn")
            nc.vector.tensor_tensor(zn[:], z[:], nn[:], A.mult)
            h = constp.tile([128, 4], F32, tag=f"h{layer}")
            nc.vector.tensor_tensor(h[:], nn[:], zn[:], A.subtract)
            return h

        h0 = gru_cell(0, lambda c: xenc_t[:69, :], [69])
        h1 = gru_cell(1, lambda c: h0[:, c:c + 1], [128] * 4)

        # decoder input tile [128, 10]: cols 0..3 = h_enc0, 4..7 = h_enc1,
        # col 8 = (skel|geo)[0:128], col 9 = (skel|geo)[128:194] then zeros.
        dec_t = constp.tile([128, 10], F32, tag="dec_t")
        nc.vector.tensor_copy(dec_t[:, 0:4], h0[:])
        nc.vector.tensor_copy(dec_t[:, 4:8], h1[:])
        nc.gpsimd.memset(dec_t[:, 9:10], 0.0)
        nc.sync.dma_start(dec_t[:, 8:9], prefix[0:128, :])
        nc.sync.dma_start(dec_t[:66, 9:10], prefix[128:194, :])

        hd0 = gru_cell(2, lambda c: dec_t[:, c:c + 1], [128] * 10)
        hd1 = gru_cell(3, lambda c: hd0[:, c:c + 1], [128] * 4)

        # --- heads ---
        # local_rot row [1,66] = h_last^T @ rotW^T + rotb
        lr_ps = psumc.tile([1, 66], F32, tag="lr")
        for c in range(4):
            nc.tensor.matmul(
                lr_ps[:], hd1[:, c:c + 1], rotw_t[:, c * 66:(c + 1) * 66],
                start=(c == 0), stop=(c == 3),
            )
        lr_row = constp.tile([1, 66], F32, tag="lr_row")
        nc.vector.tensor_tensor(lr_row[:], lr_ps[:], rotb_t[:], A.add)

        # root_vel column [3,1] = (velW^T)^T @ h_last + velb
        rv_ps = psumc.tile([3, 1], F32, tag="rv")
        for c in range(4):
            nc.tensor.matmul(
                rv_ps[:], velw_t[:, c * 3:(c + 1) * 3], hd1[:, c:c + 1],
                start=(c == 0), stop=(c == 3),
            )
        rv_col = constp.tile([3, 1], F32, tag="rv_col")
        nc.vector.tensor_tensor(rv_col[:], rv_ps[:], velb_t[:], A.add)

        # rv repeated 22x: [1,66] = rv_col^T @ s3
        rv66_ps = psumc.tile([1, 66], F32, tag="rv66")
        nc.tensor.matmul(rv66_ps[:], rv_col[:], s3_t[:], start=True, stop=True)
        gp_row = constp.tile([1, 66], F32, tag="gp_row")
        nc.vector.tensor_tensor(gp_row[:], lr_row[:], rv66_ps[:], A.mult)
        nc.sync.dma_start(gpos[:], gp_row[:])

        # offsets broadcast to all partitions: [128, 66] = ones^T @ lr_row
        offc_ps = psumc.tile([128, 66], F32, tag="offc")
        nc.tensor.matmul(offc_ps[:], ones_t[:], lr_row[:], start=True, stop=True)
        offc = constp.tile([128, 66], F32, tag="offc")
        nc.vector.tensor_copy(offc[:], offc_ps[:])

        # --- skinning over vertex tiles ---
        vstream = ctx.enter_context(tc.tile_pool(name="vstream", bufs=2))
        wbfp = ctx.enter_context(tc.tile_pool(name="wbfp", bufs=1))
        maskp = ctx.enter_context(tc.tile_pool(name="maskp", bufs=4))
        mwp = ctx.enter_context(tc.tile_pool(name="mwp", bufs=6))
        streep = ctx.enter_context(tc.tile_pool(name="streep", bufs=4))
        accp = ctx.enter_context(tc.tile_pool(name="accp", bufs=1))
        tscp = ctx.enter_context(tc.tile_pool(name="tscp", bufs=3))
        outp = ctx.enter_context(tc.tile_pool(name="outp", bufs=2))

        for t in range(n_tiles):
            base = t * P * F
            sl = lambda: slice(base, base + P * F)

            idx_t = [vstream.tile([P, F], BF16, tag=f"idx{k}", name=f"idx{k}") for k in range(K)]
            w_t = [vstream.tile([P, F], F32, tag=f"w{k}", name=f"w{k}") for k in range(K)]
            v_t = [vstream.tile([P, F], F32, tag=f"v{d}", name=f"v{d}") for d in range(3)]
            for k in range(K):
                nc.sync.dma_start(idx_t[k][:], idxT[k, sl()].rearrange("(p f) -> p f", p=P))
                nc.sync.dma_start(w_t[k][:], wT[k, sl()].rearrange("(p f) -> p f", p=P))
            for d in range(3):
                nc.sync.dma_start(v_t[d][:], vT[d, sl()].rearrange("(p f) -> p f", p=P))

            # bf16 copies of w on the Scalar engine (frees DVE cycles)
            wbf_t = [wbfp.tile([P, F], BF16, tag=f"wbf{k}", name=f"wbf{k}") for k in range(K)]
            for k in range(K):
                nc.scalar.copy(wbf_t[k][:], w_t[k][:])

            # wsum in fp32
            ws01 = accp.tile([P, F], F32, tag="ws01")
            ws = accp.tile([P, F], F32, tag="ws")
            nc.gpsimd.tensor_tensor(ws01[:], w_t[0][:], w_t[1][:], A.add)
            nc.gpsimd.tensor_tensor(ws[:], w_t[2][:], w_t[3][:], A.add)
            nc.gpsimd.tensor_tensor(ws[:], ws01[:], ws[:], A.add)

            p_t = [accp.tile([P, F], BF16, tag=f"P{d}", name=f"P{d}") for d in range(3)]
            for b in range(NB):
                mw = []
                for k in range(K):
                    m = maskp.tile([P, F], BF16, tag="mask")
                    nc.vector.tensor_scalar(m[:], idx_t[k][:], float(b), None, A.is_equal)
                    mwk = mwp.tile([P, F], BF16, tag="mw")
                    eng = nc.gpsimd if k == 3 else nc.vector
                    eng.tensor_tensor(mwk[:], m[:], wbf_t[k][:], A.mult)
                    mw.append(mwk)
                s01 = streep.tile([P, F], BF16, tag="s01")
                s23 = streep.tile([P, F], BF16, tag="s23")
                sb = streep.tile([P, F], BF16, tag="sb")
                nc.vector.tensor_tensor(s01[:], mw[0][:], mw[1][:], A.add)
                nc.vector.tensor_tensor(s23[:], mw[2][:], mw[3][:], A.add)
                nc.vector.tensor_tensor(sb[:], s01[:], s23[:], A.add)
                for d in range(3):
                    col = offc[:, 3 * b + d:3 * b + d + 1]
                    if b == 0:
                        nc.scalar.activation(p_t[d][:], sb[:], AF.Copy, scale=col)
                    else:
                        tsc = tscp.tile([P, F], BF16, tag=f"tsc{d}", name=f"tsc{d}")
                        nc.scalar.activation(tsc[:], sb[:], AF.Copy, scale=col)
                        nc.vector.tensor_tensor(p_t[d][:], p_t[d][:], tsc[:], A.add)

            for d in range(3):
                o = outp.tile([P, F], F32, tag=f"o{d}")
                nc.vector.tensor_tensor(o[:], v_t[d][:], ws[:], A.mult)
                nc.vector.tensor_tensor(o[:], o[:], p_t[d][:], A.add)
                nc.gpsimd.dma_start(outT[d, sl()].rearrange("(p f) -> p f", p=P), o[:])

    nc.compile()
    return nc


def prep_shared_inputs(rotation, velocity, params):
    """Host-side relayout of the small inputs and parameters (shared by all cores)."""
    f32 = lambda a: np.ascontiguousarray(np.asarray(a, dtype=np.float32))
    rotation = f32(rotation)
    velocity = f32(velocity)

    enc = params["enc"]
    dec = params["dec"]
    layers = [enc[0], enc[1], dec[0], dec[1]]

    shared = {}
    shared["x_enc"] = np.concatenate([rotation.reshape(-1), velocity]).astype(np.float32).reshape(69, 1)

    for i, (Wih, Whh, bih, bhh) in enumerate(layers):
        Wt = f32(Wih).T  # [in, 1536]
        if i == 2:
            Wt = Wt[DEC_PERM][:DEC_KEEP]
        shared[f"wih{i}"] = np.ascontiguousarray(Wt)
        bih = f32(bih)
        bhh = f32(bhh)
        vec = np.concatenate([(bih + bhh)[:1024], bih[1024:]])
        shared[f"b12_{i}"] = np.ascontiguousarray(vec.reshape(NG, 128).T)
        shared[f"bnh_{i}"] = np.ascontiguousarray(bhh[1024:].reshape(4, 128).T)

    shared["prefix"] = np.concatenate(
        [f32(params["skel"]).reshape(-1), f32(params["geo_emb"])]
    ).astype(np.float32).reshape(194, 1)
    shared["rotwT"] = np.ascontiguousarray(f32(params["rotW"]).T)
    shared["rotb"] = f32(params["rotb"]).reshape(1, 66)
    shared["velwT"] = np.ascontiguousarray(f32(params["velW"]).T)
    shared["velb"] = f32(params["velb"]).reshape(3, 1)
    s3 = np.zeros((3, 66), np.float32)
    s3[np.arange(66) % 3, np.arange(66)] = 1.0
    shared["s3"] = s3
    shared["ones1"] = np.ones((1, 128), np.float32)
    return shared


def prep_shard(vertices, skin_idx, skin_w, s, vs):
    sl = slice(s * vs, (s + 1) * vs)
    return {
        "vT": np.ascontiguousarray(np.asarray(vertices[sl], np.float32).T),
        "wT": np.ascontiguousarray(np.asarray(skin_w[sl], np.float32).T),
        "idxT": np.ascontiguousarray(
            np.asarray(skin_idx[sl]).T.astype(ml_dtypes.bfloat16)
        ),
    }


LAST_RESULTS = None
_NC_CACHE = {}


def _get_program(vs, f_tile):
    key = (vs, f_tile)
    if key not in _NC_CACHE:
        _NC_CACHE[key] = build_program(vs, f_tile)
    return _NC_CACHE[key]


def kernel(rotation, velocity, vertices, skin_idx, skin_w, params):
    vertices = np.asarray(vertices)
    skin_idx = np.asarray(skin_idx)
    skin_w = np.asarray(skin_w)
    n = vertices.shape[0]
    vs = n // N_CORES

    shared = prep_shared_inputs(rotation, velocity, params)
    in_maps = []
    for s in range(N_CORES):
        m = dict(shared)
        m.update(prep_shard(vertices, skin_idx, skin_w, s, vs))
        in_maps.append(m)

    nc = _get_program(vs, F_TILE)
    import os

    res = run_bass_kernel_spmd(
        nc, in_maps, list(range(N_CORES)),
        trace=bool(os.environ.get("KERNEL_TRACE")),
    )
    global LAST_RESULTS
    LAST_RESULTS = res
    outs = res.results

    vertex_positions = np.concatenate(
        [np.ascontiguousarray(outs[s]["outT"].T) for s in range(N_CORES)], axis=0
    ).astype(np.float32)
    global_positions = np.asarray(outs[0]["gpos"], np.float32).reshape(66)
    return (global_positions, vertex_positions)


# revision 31
# speedup vs baseline: 1.3844x; 1.0359x over previous
"""Trainium2 Bass kernel for nn_BaseModel_91096256348406 (GRU pose net + LBS skinning).

Host side: pure relayout/sharding (transpose planes, dtype casts, weight
pre-transposes). Device side (SPMD x8 NeuronCores): 2-layer GRU encoder +
2-layer GRU decoder (single step from h0=0, so gh == bhh and h' = (1-z)*n),
output heads, then per-vertex skinning:

    out[n,d] = v[n,d]*wsum[n] + sum_k w[n,k]*off[idx[n,k],d]

The 22-entry table lookup is computed as 22 per-bone masked accumulations,
split across three engines: bf16 equality masks (4x) and weight products /
pair-sums (2x) on the Vector engine, the per-bone scale S_b*off_bd on the
Scalar engine (ACT Copy with per-partition scale), the bf16 P accumulation
adds back on Vector at 2x, and wsum plus k=3's weight product on GPSIMD.
scalar_tensor_tensor was measured 1x-only and TensorScalarPtr is illegal on
Pool, which is what drove this split (see TimelineSim experiments). GRU
weights stream as 512-column slabs (57 DMA issues instead of 228) to keep
the SP descriptor queue off the critical path.
"""

import sys

sys.path.insert(0, "/opt/trn_rl_repo")

import numpy as np
import ml_dtypes
from contextlib import ExitStack

import concourse.bass as bass
import concourse.mybir as mybir
from concourse import bacc, tile
from concourse.bass_utils import run_bass_kernel_spmd

F32 = mybir.dt.float32
BF16 = mybir.dt.bfloat16
A = mybir.AluOpType
AF = mybir.ActivationFunctionType

N_CORES = 8
N_VERTS = 4194304
VS = N_VERTS // N_CORES  # 524288 vertices per core
P = 128
F_TILE = 1024
NB = 22  # bones
K = 4    # influences per vertex
H = 512
NG = 12  # 1536 gate rows = 12 chunks of 128

# decoder layer-0 input permutation (host side): reorder dec_in from
# [prev(69) skel(66) geo(128) h0(512) h1(512)] to
# [h0 h1 skel geo prev-zeros], then drop the trailing 7 all-zero rows so the
# contraction is 10 chunks of 128 (rows 1218..1279 are zeros; the SBUF tile
# columns they multiply are memset to 0).
DEC_PERM = np.concatenate(
    [np.arange(263, 1287), np.arange(69, 263), np.arange(0, 69)]
)
DEC_KEEP = 1280


def build_program(vs=VS, f_tile=F_TILE):
    """Build the SPMD single-core Bass program for a shard of `vs` vertices."""
    assert vs % (P * f_tile) == 0
    n_tiles = vs // (P * f_tile)
    F = f_tile

    nc = bacc.Bacc(None, target_bir_lowering=False)

    # ---- external inputs ----
    vT = nc.dram_tensor("vT", [3, vs], F32, kind="ExternalInput")
    wT = nc.dram_tensor("wT", [4, vs], F32, kind="ExternalInput")
    idxT = nc.dram_tensor("idxT", [4, vs], BF16, kind="ExternalInput")

    x_enc = nc.dram_tensor("x_enc", [69, 1], F32, kind="ExternalInput")
    wih = [
        nc.dram_tensor("wih0", [69, 1536], F32, kind="ExternalInput"),
        nc.dram_tensor("wih1", [512, 1536], F32, kind="ExternalInput"),
        nc.dram_tensor("wih2", [DEC_KEEP, 1536], F32, kind="ExternalInput"),
        nc.dram_tensor("wih3", [512, 1536], F32, kind="ExternalInput"),
    ]
    b12 = [nc.dram_tensor(f"b12_{i}", [128, NG], F32, kind="ExternalInput") for i in range(4)]
    bnh = [nc.dram_tensor(f"bnh_{i}", [128, 4], F32, kind="ExternalInput") for i in range(4)]
    prefix = nc.dram_tensor("prefix", [194, 1], F32, kind="ExternalInput")
    rotwT = nc.dram_tensor("rotwT", [512, 66], F32, kind="ExternalInput")
    rotb = nc.dram_tensor("rotb", [1, 66], F32, kind="ExternalInput")
    velwT = nc.dram_tensor("velwT", [512, 3], F32, kind="ExternalInput")
    velb = nc.dram_tensor("velb", [3, 1], F32, kind="ExternalInput")
    s3 = nc.dram_tensor("s3", [3, 66], F32, kind="ExternalInput")
    ones1 = nc.dram_tensor("ones1", [1, 128], F32, kind="ExternalInput")

    # ---- external outputs ----
    outT = nc.dram_tensor("outT", [3, vs], F32, kind="ExternalOutput")
    gpos = nc.dram_tensor("gpos", [1, 66], F32, kind="ExternalOutput")

    with tile.TileContext(nc) as tc, ExitStack() as ctx:
        constp = ctx.enter_context(tc.tile_pool(name="constp", bufs=1))
        wpool = ctx.enter_context(tc.tile_pool(name="wpool", bufs=11))
        grup = ctx.enter_context(tc.tile_pool(name="grup", bufs=2))
        psum = ctx.enter_context(tc.tile_pool(name="psum", bufs=2, space="PSUM"))
        psumc = ctx.enter_context(tc.tile_pool(name="psumc", bufs=1, space="PSUM"))

        # --- small constants into SBUF ---
        xenc_t = constp.tile([128, 1], F32, tag="xenc")
        nc.sync.dma_start(xenc_t[:69, :], x_enc[:, :])
        b12_t = []
        bnh_t = []
        for i in range(4):
            bt = constp.tile([128, NG], F32, tag=f"b12_{i}")
            nc.sync.dma_start(bt[:], b12[i][:])
            b12_t.append(bt)
            nt = constp.tile([128, 4], F32, tag=f"bnh_{i}")
            nc.sync.dma_start(nt[:], bnh[i][:])
            bnh_t.append(nt)
        rotw_t = constp.tile([128, 4 * 66], F32, tag="rotw")
        for c in range(4):
            nc.sync.dma_start(rotw_t[:, c * 66:(c + 1) * 66], rotwT[c * 128:(c + 1) * 128, :])
        rotb_t = constp.tile([1, 66], F32, tag="rotb")
        nc.sync.dma_start(rotb_t[:], rotb[:])
        velw_t = constp.tile([128, 4 * 3], F32, tag="velw")
        for c in range(4):
            nc.sync.dma_start(velw_t[:, c * 3:(c + 1) * 3], velwT[c * 128:(c + 1) * 128, :])
        velb_t = constp.tile([3, 1], F32, tag="velb")
        nc.sync.dma_start(velb_t[:], velb[:])
        s3_t = constp.tile([3, 66], F32, tag="s3")
        nc.sync.dma_start(s3_t[:], s3[:])
        ones_t = constp.tile([1, 128], F32, tag="ones1")
        nc.sync.dma_start(ones_t[:], ones1[:])

        # --- GRU: one cell per layer, h_prev = 0 => gh = bhh, h' = (1-z)*n ---
        def gru_cell(layer, rhs_cols, chunk_sizes):
            n_chunks = len(chunk_sizes)
            pg = psum.tile([128, NG], F32, tag="gates", name="pg")
            # 512-col weight slabs: 1 DMA per (chunk, slab) instead of per
            # (chunk, m) — 57 total issues instead of 228. m loops within a
            # resident slab so each PSUM column's accumulation group closes
            # before the next opens.
            for g in range(3):
                slabs = []
                row0 = 0
                for c in range(n_chunks):
                    C = chunk_sizes[c]
                    st = wpool.tile([128, 512], F32, tag="wslab", name="wslab")
                    nc.sync.dma_start(
                        st[:C, :], wih[layer][row0:row0 + C, g * 512:(g + 1) * 512]
                    )
                    row0 += C
                    slabs.append((st, C))
                for mi in range(4):
                    m = g * 4 + mi
                    for c in range(n_chunks):
                        st, C = slabs[c]
                        nc.tensor.matmul(
                            pg[:, m:m + 1],
                            st[:C, mi * 128:(mi + 1) * 128],
                            rhs_cols(c),
                            start=(c == 0),
                            stop=(c == n_chunks - 1),
                        )
            g = grup.tile([128, NG], F32, tag="g")
            nc.vector.tensor_tensor(g[:], pg[:], b12_t[layer][:], A.add)
            r = grup.tile([128, 4], F32, tag="r")
            nc.scalar.activation(r[:], g[:, 0:4], AF.Sigmoid)
            z = grup.tile([128, 4], F32, tag="z")
            nc.scalar.activation(z[:], g[:, 4:8], AF.Sigmoid)
            npre = grup.tile([128, 4], F32, tag="npre")
            nc.vector.tensor_tensor(npre[:], r[:], bnh_t[layer][:], A.mult)
            nc.vector.tensor_tensor(npre[:], g[:, 8:12], npre[:], A.add)
            nn = grup.tile([128, 4], F32, tag="nn")
            nc.scalar.activation(nn[:], npre[:], AF.Tanh)
            zn = grup.tile([128, 4], F32, tag="zn")
            nc.vector.tensor_tensor(zn[:], z[:], nn[:], A.mult)
            h = constp.tile([128, 4], F32, tag=f"h{layer}")
            nc.vector.tensor_tensor(h[:], nn[:], zn[:], A.subtract)
            return h

        h0 = gru_cell(0, lambda c: xenc_t[:69, :], [69])
        h1 = gru_cell(1, lambda c: h0[:, c:c + 1], [128] * 4)

        # decoder input tile [128, 10]: cols 0..3 = h_enc0, 4..7 = h_enc1,
        # col 8 = (skel|geo)[0:128], col 9 = (skel|geo)[128:194] then zeros.
        dec_t = constp.tile([128, 10], F32, tag="dec_t")
        nc.vector.tensor_copy(dec_t[:, 0:4], h0[:])
        nc.vector.tensor_copy(dec_t[:, 4:8], h1[:])
        nc.gpsimd.memset(dec_t[:, 9:10], 0.0)
        nc.sync.dma_start(dec_t[:, 8:9], prefix[0:128, :])
        nc.sync.dma_start(dec_t[:66, 9:10], prefix[128:194, :])

        hd0 = gru_cell(2, lambda c: dec_t[:, c:c + 1], [128] * 10)
        hd1 = gru_cell(3, lambda c: hd0[:, c:c + 1], [128] * 4)

        # --- heads ---
        # local_rot row [1,66] = h_last^T @ rotW^T + rotb
        lr_ps = psumc.tile([1, 66], F32, tag="lr")
        for c in range(4):
            nc.tensor.matmul(
                lr_ps[:], hd1[:, c:c + 1], rotw_t[:, c * 66:(c + 1) * 66],
                start=(c == 0), stop=(c == 3),
            )
        lr_row = constp.tile([1, 66], F32, tag="lr_row")
        nc.vector.tensor_tensor(lr_row[:], lr_ps[:], rotb_t[:], A.add)

        # root_vel column [3,1] = (velW^T)^T @ h_last + velb
        rv_ps = psumc.tile([3, 1], F32, tag="rv")
        for c in range(4):
            nc.tensor.matmul(
                rv_ps[:], velw_t[:, c * 3:(c + 1) * 3], hd1[:, c:c + 1],
                start=(c == 0), stop=(c == 3),
            )
        rv_col = constp.tile([3, 1], F32, tag="rv_col")
        nc.vector.tensor_tensor(rv_col[:], rv_ps[:], velb_t[:], A.add)

        # rv repeated 22x: [1,66] = rv_col^T @ s3
        rv66_ps = psumc.tile([1, 66], F32, tag="rv66")
        nc.tensor.matmul(rv66_ps[:], rv_col[:], s3_t[:], start=True, stop=True)
        gp_row = constp.tile([1, 66], F32, tag="gp_row")
        nc.vector.tensor_tensor(gp_row[:], lr_row[:], rv66_ps[:], A.mult)
        nc.sync.dma_start(gpos[:], gp_row[:])

        # offsets broadcast to all partitions: [128, 66] = ones^T @ lr_row
        lr2_row = constp.tile([1, 66], F32, tag="lr2_row")
        nc.vector.tensor_copy(lr2_row[:, 63:66], lr_row[:, 63:66])
        lr3 = lr_row[:, 0:63].rearrange("p (b t) -> p b t", t=3)
        lr23 = lr2_row[:, 0:63].rearrange("p (b t) -> p b t", t=3)
        for d in range(3):
            nc.vector.tensor_scalar(
                lr23[:, :, d], lr3[:, :, d], lr_row[:, 63 + d:64 + d], None, A.subtract
            )
        offc_ps = psumc.tile([128, 66], F32, tag="offc")
        nc.tensor.matmul(offc_ps[:], ones_t[:], lr2_row[:], start=True, stop=True)
        offc = constp.tile([128, 66], F32, tag="offc")
        nc.vector.tensor_copy(offc[:], offc_ps[:])

        # --- skinning over vertex tiles ---
        vstream = ctx.enter_context(tc.tile_pool(name="vstream", bufs=2))
        wbfp = ctx.enter_context(tc.tile_pool(name="wbfp", bufs=1))
        maskp = ctx.enter_context(tc.tile_pool(name="maskp", bufs=4))
        mwp = ctx.enter_context(tc.tile_pool(name="mwp", bufs=6))
        streep = ctx.enter_context(tc.tile_pool(name="streep", bufs=4))
        accp = ctx.enter_context(tc.tile_pool(name="accp", bufs=1))
        tscp = ctx.enter_context(tc.tile_pool(name="tscp", bufs=3))
        outp = ctx.enter_context(tc.tile_pool(name="outp", bufs=2))

        for t in range(n_tiles):
            base = t * P * F
            sl = lambda: slice(base, base + P * F)

            idx_t = [vstream.tile([P, F], BF16, tag=f"idx{k}", name=f"idx{k}") for k in range(K)]
            w_t = [vstream.tile([P, F], F32, tag=f"w{k}", name=f"w{k}") for k in range(K)]
            v_t = [vstream.tile([P, F], F32, tag=f"v{d}", name=f"v{d}") for d in range(3)]
            for k in range(K):
                nc.sync.dma_start(idx_t[k][:], idxT[k, sl()].rearrange("(p f) -> p f", p=P))
                nc.sync.dma_start(w_t[k][:], wT[k, sl()].rearrange("(p f) -> p f", p=P))
            for d in range(3):
                nc.sync.dma_start(v_t[d][:], vT[d, sl()].rearrange("(p f) -> p f", p=P))

            # bf16 copies of w on the Scalar engine (frees DVE cycles)
            wbf_t = [wbfp.tile([P, F], BF16, tag=f"wbf{k}", name=f"wbf{k}") for k in range(K)]
            for k in range(K):
                nc.scalar.copy(wbf_t[k][:], w_t[k][:])

            # wsum in fp32
            ws01 = accp.tile([P, F], F32, tag="ws01")
            ws = accp.tile([P, F], F32, tag="ws")
            nc.gpsimd.tensor_tensor(ws01[:], w_t[0][:], w_t[1][:], A.add)
            nc.gpsimd.tensor_tensor(ws[:], w_t[2][:], w_t[3][:], A.add)
            nc.gpsimd.tensor_tensor(ws[:], ws01[:], ws[:], A.add)

            p_t = [accp.tile([P, F], BF16, tag=f"P{d}", name=f"P{d}") for d in range(3)]
            for d in range(3):
                nc.scalar.activation(
                    p_t[d][:], ws[:], AF.Copy, scale=offc[:, 63 + d:64 + d]
                )
            for b in range(NB - 1):
                mw = []
                for k in range(K):
                    m = maskp.tile([P, F], BF16, tag="mask")
                    nc.vector.tensor_scalar(m[:], idx_t[k][:], float(b), None, A.is_equal)
                    mwk = mwp.tile([P, F], BF16, tag="mw")
                    eng = nc.gpsimd if k == 3 else nc.vector
                    eng.tensor_tensor(mwk[:], m[:], wbf_t[k][:], A.mult)
                    mw.append(mwk)
                s01 = streep.tile([P, F], BF16, tag="s01")
                s23 = streep.tile([P, F], BF16, tag="s23")
                sb = streep.tile([P, F], BF16, tag="sb")
                nc.vector.tensor_tensor(s01[:], mw[0][:], mw[1][:], A.add)
                nc.vector.tensor_tensor(s23[:], mw[2][:], mw[3][:], A.add)
                nc.vector.tensor_tensor(sb[:], s01[:], s23[:], A.add)
                for d in range(3):
                    col = offc[:, 3 * b + d:3 * b + d + 1]
                    tsc = tscp.tile([P, F], BF16, tag=f"tsc{d}", name=f"tsc{d}")
                    nc.scalar.activation(tsc[:], sb[:], AF.Copy, scale=col)
                    nc.vector.tensor_tensor(p_t[d][:], p_t[d][:], tsc[:], A.add)

            for d in range(3):
                o = outp.tile([P, F], F32, tag=f"o{d}")
                nc.vector.tensor_tensor(o[:], v_t[d][:], ws[:], A.mult)
                nc.vector.tensor_tensor(o[:], o[:], p_t[d][:], A.add)
                nc.gpsimd.dma_start(outT[d, sl()].rearrange("(p f) -> p f", p=P), o[:])

    nc.compile()
    return nc


def prep_shared_inputs(rotation, velocity, params):
    """Host-side relayout of the small inputs and parameters (shared by all cores)."""
    f32 = lambda a: np.ascontiguousarray(np.asarray(a, dtype=np.float32))
    rotation = f32(rotation)
    velocity = f32(velocity)

    enc = params["enc"]
    dec = params["dec"]
    layers = [enc[0], enc[1], dec[0], dec[1]]

    shared = {}
    shared["x_enc"] = np.concatenate([rotation.reshape(-1), velocity]).astype(np.float32).reshape(69, 1)

    for i, (Wih, Whh, bih, bhh) in enumerate(layers):
        Wt = f32(Wih).T  # [in, 1536]
        if i == 2:
            Wt = Wt[DEC_PERM][:DEC_KEEP]
        shared[f"wih{i}"] = np.ascontiguousarray(Wt)
        bih = f32(bih)
        bhh = f32(bhh)
        vec = np.concatenate([(bih + bhh)[:1024], bih[1024:]])
        shared[f"b12_{i}"] = np.ascontiguousarray(vec.reshape(NG, 128).T)
        shared[f"bnh_{i}"] = np.ascontiguousarray(bhh[1024:].reshape(4, 128).T)

    shared["prefix"] = np.concatenate(
        [f32(params["skel"]).reshape(-1), f32(params["geo_emb"])]
    ).astype(np.float32).reshape(194, 1)
    shared["rotwT"] = np.ascontiguousarray(f32(params["rotW"]).T)
    shared["rotb"] = f32(params["rotb"]).reshape(1, 66)
    shared["velwT"] = np.ascontiguousarray(f32(params["velW"]).T)
    shared["velb"] = f32(params["velb"]).reshape(3, 1)
    s3 = np.zeros((3, 66), np.float32)
    s3[np.arange(66) % 3, np.arange(66)] = 1.0
    shared["s3"] = s3
    shared["ones1"] = np.ones((1, 128), np.float32)
    return shared


def prep_shard(vertices, skin_idx, skin_w, s, vs):
    sl = slice(s * vs, (s + 1) * vs)
    return {
        "vT": np.ascontiguousarray(np.asarray(vertices[sl], np.float32).T),
        "wT": np.ascontiguousarray(np.asarray(skin_w[sl], np.float32).T),
        "idxT": np.ascontiguousarray(
            np.asarray(skin_idx[sl]).T.astype(ml_dtypes.bfloat16)
        ),
    }


LAST_RESULTS = None
_NC_CACHE = {}


def _get_program(vs, f_tile):
    key = (vs, f_tile)
    if key not in _NC_CACHE:
        _NC_CACHE[key] = build_program(vs, f_tile)
    return _NC_CACHE[key]


def kernel(rotation, velocity, vertices, skin_idx, skin_w, params):
    vertices = np.asarray(vertices)
    skin_idx = np.asarray(skin_idx)
    skin_w = np.asarray(skin_w)
    n = vertices.shape[0]
    vs = n // N_CORES

    shared = prep_shared_inputs(rotation, velocity, params)
    in_maps = []
    for s in range(N_CORES):
        m = dict(shared)
        m.update(prep_shard(vertices, skin_idx, skin_w, s, vs))
        in_maps.append(m)

    nc = _get_program(vs, F_TILE)
    import os

    res = run_bass_kernel_spmd(
        nc, in_maps, list(range(N_CORES)),
        trace=bool(os.environ.get("KERNEL_TRACE")),
    )
    global LAST_RESULTS
    LAST_RESULTS = res
    outs = res.results

    vertex_positions = np.concatenate(
        [np.ascontiguousarray(outs[s]["outT"].T) for s in range(N_CORES)], axis=0
    ).astype(np.float32)
    global_positions = np.asarray(outs[0]["gpos"], np.float32).reshape(66)
    return (global_positions, vertex_positions)
